# revision 2
# baseline (speedup 1.0000x reference)
"""ClusterQuantizer (VQ codebook) Trainium2 Bass kernel.

Data-parallel over 8 NeuronCores: z [16,64,64,256] is sharded along B into
8 shards of [2,64,64,256] -> [8192,256] rows; the [1024,256] codebook is
replicated. Per core, per 128-row tile:
  - PE (fp32, K=128 chunks accumulated in PSUM in order) computes
    2*z@cb^T — bit-identical to the XLA einsum the reference uses.
  - DVE scalar_tensor_tensor computes nd = fl(fl(-e2 - z2) + 2ze), which is
    exactly -dist with the reference's rounding structure, then max8 +
    max_index give argmin with lowest-index tie-break (first occurrence).
  - GPSIMD indirect DMA gathers codebook rows (z_q), computes
    diff = z_q - z and z_q_st = z + diff.
  - ACT accumulates sum(diff^2) per row for the losses.
Scalar finalization (losses, perplexity, histogram) happens on host from
per-core partials; it is O(1e5) work vs the 34 GFLOP on device.
"""
import os
import numpy as np

import concourse.bacc as bacc
import concourse.bass as bass
import concourse.tile as tile
import concourse.mybir as mybir

N_CLUSTERS = 1024
EMBED_DIM = 256
BETA = np.float32(0.25)
EPS = 1e-05

B, V, P = 16, 64, 64
N_CORES = 8
ROWS_PER_CORE = B * V * P // N_CORES          # 8192
NTILES = ROWS_PER_CORE // 128                 # 64

_cache = {}


def _build_nc(ntiles=NTILES, rows=ROWS_PER_CORE):
    nc = bacc.Bacc("TRN2", target_bir_lowering=False, debug=False,
                   num_devices=N_CORES)
    f32, u32 = mybir.dt.float32, mybir.dt.uint32

    d_zT = nc.dram_tensor("zT", [EMBED_DIM, rows], f32, kind="ExternalInput")
    d_z = nc.dram_tensor("z", [rows, EMBED_DIM], f32, kind="ExternalInput")
    d_z2negb = nc.dram_tensor("z2negb", [128, ntiles], f32, kind="ExternalInput")
    d_cb2T = nc.dram_tensor("cb2T", [EMBED_DIM, N_CLUSTERS], f32, kind="ExternalInput")
    d_e2negb = nc.dram_tensor("e2negb", [128, N_CLUSTERS], f32, kind="ExternalInput")
    d_cb = nc.dram_tensor("cb", [N_CLUSTERS, EMBED_DIM], f32, kind="ExternalInput")

    d_zqst = nc.dram_tensor("zqst", [rows, EMBED_DIM], f32, kind="ExternalOutput")
    d_idxb = nc.dram_tensor("idxb", [128, ntiles], u32, kind="ExternalOutput")
    d_lossb = nc.dram_tensor("lossb", [128, ntiles], f32, kind="ExternalOutput")

    with tile.TileContext(nc) as tc:
        with (
            tc.tile_pool(name="const", bufs=1) as cpool,
            tc.tile_pool(name="zin", bufs=4) as zpool,
            tc.tile_pool(name="nd", bufs=4) as ndpool,
            tc.tile_pool(name="small", bufs=4) as spool,
            tc.tile_pool(name="q", bufs=4) as qpool,
            tc.tile_pool(name="acc", bufs=1) as apool,
            tc.tile_pool(name="psum", bufs=3, space="PSUM") as psum,
        ):
            cb2T_sb = cpool.tile([128, 2 * N_CLUSTERS], f32)
            e2negb_sb = cpool.tile([128, N_CLUSTERS], f32)
            z2negb_sb = cpool.tile([128, ntiles], f32)
            nc.sync.dma_start(cb2T_sb[:, 0:N_CLUSTERS], d_cb2T[0:128, :])
            nc.sync.dma_start(cb2T_sb[:, N_CLUSTERS:], d_cb2T[128:256, :])
            nc.sync.dma_start(e2negb_sb[:], d_e2negb[:])
            nc.sync.dma_start(z2negb_sb[:], d_z2negb[:])

            idx_acc = apool.tile([128, ntiles], u32)
            loss_acc = apool.tile([128, ntiles], f32)

            for i in range(ntiles):
                # ---- loads ----
                zT_sb = zpool.tile([128, 256], f32, tag="zT")
                nc.sync.dma_start(zT_sb[:, 0:128], d_zT[0:128, i*128:(i+1)*128])
                nc.sync.dma_start(zT_sb[:, 128:256], d_zT[128:256, i*128:(i+1)*128])
                z_sb = zpool.tile([128, 256], f32, tag="z")
                nc.sync.dma_start(z_sb[:], d_z[i*128:(i+1)*128, :])

                # ---- matmul: ps = 2 * z @ cb^T  (fp32, K chunk 0 then 1) ----
                ps = psum.tile([128, N_CLUSTERS], f32)
                for kh in range(2):
                    nc.tensor.matmul(ps[:, kh*512:(kh+1)*512], zT_sb[:, 0:128],
                                     cb2T_sb[:, kh*512:(kh+1)*512],
                                     start=True, stop=False)
                    nc.tensor.matmul(ps[:, kh*512:(kh+1)*512], zT_sb[:, 128:256],
                                     cb2T_sb[:, N_CLUSTERS+kh*512:N_CLUSTERS+(kh+1)*512],
                                     start=False, stop=True)

                # ---- nd = (-e2 + -z2) + 2ze   (exactly -dist) ----
                nd_sb = ndpool.tile([128, N_CLUSTERS], f32, tag="nd")
                nc.vector.scalar_tensor_tensor(
                    out=nd_sb[:], in0=e2negb_sb[:], scalar=z2negb_sb[:, i:i+1],
                    in1=ps[:], op0=mybir.AluOpType.add, op1=mybir.AluOpType.add)

                # ---- argmax of nd = argmin of dist (ties -> lowest index) ----
                m8 = spool.tile([128, 8], f32, tag="m8")
                i8 = spool.tile([128, 8], u32, tag="i8")
                nc.vector.max(m8[:], nd_sb[:])
                nc.vector.max_index(i8[:], m8[:], nd_sb[:])
                nc.gpsimd.tensor_copy(idx_acc[:, i:i+1], i8[:, 0:1])

                # ---- gather z_q = cb[idx] ----
                zq_sb = qpool.tile([128, 256], f32, tag="zq")
                nc.gpsimd.indirect_dma_start(
                    out=zq_sb[:], out_offset=None, in_=d_cb[:],
                    in_offset=bass.IndirectOffsetOnAxis(ap=i8[:, 0:1], axis=0))

                # ---- diff = zq - z ; zqst = z + diff ; loss += diff^2 ----
                diff_sb = qpool.tile([128, 256], f32, tag="diff")
                nc.gpsimd.tensor_tensor(diff_sb[:], zq_sb[:], z_sb[:],
                                        op=mybir.AluOpType.subtract)
                zqst_sb = qpool.tile([128, 256], f32, tag="zqst")
                nc.gpsimd.tensor_tensor(zqst_sb[:], z_sb[:], diff_sb[:],
                                        op=mybir.AluOpType.add)
                nc.scalar.activation(
                    diff_sb[:], diff_sb[:], mybir.ActivationFunctionType.Square,
                    accum_out=loss_acc[:, i:i+1])

                nc.sync.dma_start(d_zqst[i*128:(i+1)*128, :], zqst_sb[:])

            nc.sync.dma_start(d_idxb[:], idx_acc[:])
            nc.sync.dma_start(d_lossb[:], loss_acc[:])

    nc.compile()
    return nc


def _get_runner():
    if "runner" in _cache:
        return _cache["runner"]
    from concourse.bass_utils import run_bass_kernel_spmd
    nc = _build_nc()

    def run(in_maps):
        return run_bass_kernel_spmd(nc, in_maps, core_ids=list(range(N_CORES))).results

    _cache["runner"] = run
    return run


def kernel(z, codebook):
    z = np.asarray(z, dtype=np.float32)
    codebook = np.ascontiguousarray(np.asarray(codebook, dtype=np.float32))
    zf = z.reshape(-1, EMBED_DIM)                       # [65536, 256]
    n_total = zf.shape[0]

    # host prep (cheap; ze/dist precision-critical parts are on device)
    cb2T = np.ascontiguousarray((np.float32(2.0) * codebook).T)   # exact *2
    e2 = (codebook * codebook).sum(-1, dtype=np.float32)
    e2negb = np.ascontiguousarray(np.broadcast_to(-e2[None, :], (128, N_CLUSTERS)))
    z2 = np.einsum("nd,nd->n", zf, zf, dtype=np.float32).astype(np.float32)

    in_maps = []
    for c in range(N_CORES):
        sl = slice(c * ROWS_PER_CORE, (c + 1) * ROWS_PER_CORE)
        z_shard = zf[sl]                                # [8192, 256] view
        zT = np.ascontiguousarray(z_shard.T)            # [256, 8192]
        z2negb = np.ascontiguousarray((-z2[sl]).reshape(NTILES, 128).T)
        in_maps.append({
            "zT": zT, "z": np.ascontiguousarray(z_shard), "z2negb": z2negb,
            "cb2T": cb2T, "e2negb": e2negb, "cb": codebook,
        })

    results = _get_runner()(in_maps)

    zqst = np.concatenate([r["zqst"] for r in results], axis=0).reshape(z.shape)
    idx = np.concatenate(
        [r["idxb"].T.reshape(-1) for r in results]).astype(np.int32)
    indices = idx.reshape(z.shape[:-1])

    loss_sum = np.float64(0.0)
    for r in results:
        loss_sum += r["lossb"].astype(np.float64).sum()
    n_elems = n_total * EMBED_DIM
    codebook_loss = np.float32(loss_sum / n_elems)
    commitment_loss = codebook_loss
    cluster_loss = np.float32(codebook_loss + np.float32(BETA * commitment_loss))

    counts = np.bincount(idx, minlength=N_CLUSTERS).astype(np.float64)
    probs = counts / (counts.sum() + EPS)
    perplexity = np.float32(np.exp(-np.sum(probs * np.log(probs + EPS))))

    return (zqst, commitment_loss, codebook_loss, cluster_loss, perplexity,
            indices)
